# revision 1
# baseline (speedup 1.0000x reference)
"""Trainium2 Bass kernel for nn_DecoderFactoredLSTM (v2).

Factored-LSTM decoder:
  emb = B_w[captions]                                   [B,T,E] -> tokens [T*B, E]
  u   = emb @ (V^T S^T U^T) + bias                      [T*B, 4H]   (gate pre-activations)
  recurrence over T=40 steps (LSTM, no tanh on c for h)
  out = hiddens @ C_w^T + C_b                           [T*B, V]

Sharding: recurrence + pre-projections replicated on all 8 cores; the
vocab projection (dominant FLOPs) sharded 8-way over vocab columns.

v2 structure (single fused pipeline, bf16 matmuls):
  phase A: gather+transpose emb into SBUF (bf16), M = V^T S^T U^T
           (f32r), u = embT^T @ M + bias -> DRAM (bf16, token-major)
  phase B: 40 recurrence steps with the vocab projection interleaved
           (vocab tile (t-2)//2 split across each step pair), h kept in
           SBUF as a 3-slot ring of transposed bf16 pair-tiles.

Column layout for gates is GATE-MAJOR everywhere (u, W, psum):
  col(g, h) = g * H + h,  g in {i, f, o, c~}
so sigmoid runs as one [64, 3H] op and tanh as one [64, H] op per step.
"""

import sys

if "/opt/trn_rl_repo" not in sys.path:
    sys.path.insert(0, "/opt/trn_rl_repo")

import ml_dtypes
import numpy as np

import concourse.bass as bass
import concourse.mybir as mybir
import concourse.tile as tile
from concourse import bacc
from concourse.bass import ts
from concourse.bass_utils import run_bass_kernel_spmd
from concourse.masks import make_identity

B, T, E, H, F, V = 64, 40, 512, 1024, 512, 32000
NCORES = 8
VS = V // NCORES  # vocab slice per core: 4000
TOK = T * B  # 2560 tokens
MT = TOK // 128  # 20 token tiles
NV = VS // 8  # 500 vocab cols per chunk
F32 = mybir.dt.float32
F32R = mybir.dt.float32r
BF16 = mybir.dt.bfloat16
FP8 = mybir.dt.float8e4
SIG = mybir.ActivationFunctionType.Sigmoid
TANH = mybir.ActivationFunctionType.Tanh
DR = mybir.MatmulPerfMode.DoubleRow

# fp8 scaling: W stored *8, h cast *16; gate psum compensated by /128.
W_SCALE = 8.0
H_SCALE = 16.0
INV_SCALE = 1.0 / (W_SCALE * H_SCALE)


def _build():
    nc = bacc.Bacc(None, target_bir_lowering=False, debug=False)

    with tile.TileContext(nc) as tc:
        cap_d = nc.declare_dram_parameter("cap", [TOK, 1], mybir.dt.int32, isOutput=False)
        Bw_d = nc.declare_dram_parameter("Bw", [V, E], F32, isOutput=False)
        Vg_d = nc.declare_dram_parameter("Vg", [4, F, E], F32R, isOutput=False)
        SgT_d = nc.declare_dram_parameter("SgT", [4, F, F], F32R, isOutput=False)
        UgT_d = nc.declare_dram_parameter("UgT", [4, F, H], F32R, isOutput=False)
        W_d = nc.declare_dram_parameter("Wmov", [128, 4, 2, 4 * H], FP8, isOutput=False)
        Wlo_d = nc.declare_dram_parameter("Wlo", [128, 4, 2, 4 * H], FP8, isOutput=False)
        ub_d = nc.declare_dram_parameter("ubias", [128, 4 * H], BF16, isOutput=False)
        CT_d = nc.declare_dram_parameter("CT", [H, VS], BF16, isOutput=False)
        Cb_d = nc.declare_dram_parameter("Cb", [128, VS], BF16, isOutput=False)

        out_d = nc.declare_dram_parameter("out", [TOK, VS], F32, isOutput=True)

        u_d = nc.dram_tensor("u", [TOK, 4 * H], BF16)  # token-major gate preacts

        with (
            tc.tile_pool(name="const", bufs=1) as const,
            tc.tile_pool(name="pers", bufs=1) as pers,
        ):
            id128b = const.tile([128, 128], BF16, tag="id128b")
            make_identity(nc, id128b)
            id64b = const.tile([64, 64], BF16, tag="id64b")
            make_identity(nc, id64b)

            # W (fp8 DoubleRow layout, gate-major cols) persistent.
            # Split into hi + lo fp8 planes (both at the same 8x scale) so
            # the W quantization error drops to ~0.4%; both accumulate into
            # the same psum group.
            W8 = pers.tile([128, 4, 2, 4 * H], FP8, tag="w8")
            nc.scalar.dma_start(W8[:], W_d[:])
            W8lo = pers.tile([128, 4, 2, 4 * H], FP8, tag="w8lo")
            nc.scalar.dma_start(W8lo[:], Wlo_d[:])

            # ================= phase A =================
            phA = tc.tile_pool(name="phA", bufs=1)
            pA = phA.__enter__()
            phAm = tc.tile_pool(name="phAm", bufs=2, space="PSUM")
            pAm = phAm.__enter__()
            phAt = tc.tile_pool(name="phAt", bufs=2, space="PSUM")
            pAt = phAt.__enter__()

            idx_all = pA.tile([128, MT], mybir.dt.int32, tag="idx")
            nc.sync.dma_start(idx_all[:], cap_d[:].rearrange("(m p) o -> p (m o)", p=128))
            ub16 = pA.tile([128, 4 * H], BF16, tag="ub16")
            nc.sync.dma_start(ub16[:], ub_d[:])

            emb16 = pA.tile([128, 4, TOK], BF16, tag="emb16")
            mcat16 = pA.tile([128, 4, 4 * H], BF16, tag="mcat16")

            def gather_tile(m):
                g_t = pA.tile([128, E], F32, tag=f"g{m % 3}", name=f"g{m % 3}")
                nc.gpsimd.indirect_dma_start(
                    out=g_t[:],
                    out_offset=None,
                    in_=Bw_d[:],
                    in_offset=bass.IndirectOffsetOnAxis(ap=idx_all[:, m : m + 1], axis=0),
                )
                g16 = pA.tile([128, E], BF16, tag=f"c{m % 2}", name=f"c{m % 2}")
                nc.vector.tensor_copy(g16[:], g_t[:])
                for e in range(4):
                    tp = pAt.tile([128, 128], BF16, tag="tp")
                    nc.tensor.transpose(tp[:], g16[:, ts(e, 128)], id128b[:])
                    nc.vector.tensor_copy(emb16[:, e, ts(m, 128)], tp[:])

            def mbuild_gate(g):
                # M = V^T S^T U^T (f32r), output cast to bf16, gate-major cols
                vg = pA.tile([128, 4, E], F32R, tag="vg")
                nc.sync.dma_start(vg[:], Vg_d[g].rearrange("(ko ki) e -> ki ko e", ki=128))
                sgT = pA.tile([128, 4, F], F32R, tag="sgT")
                nc.sync.dma_start(
                    sgT[:], SgT_d[g].rearrange("(ko ki) f -> ki ko f", ki=128)
                )
                ugT = pA.tile([128, 4, H], F32R, tag="ugT")
                nc.sync.dma_start(
                    ugT[:], UgT_d[g].rearrange("(ko ki) h -> ki ko h", ki=128)
                )
                pt = pA.tile([128, 4, E], F32R, tag="pt")
                for fp in range(4):
                    ps = pAm.tile([128, E], F32, tag="mp")
                    for k in range(4):
                        nc.tensor.matmul(
                            ps[:],
                            lhsT=sgT[:, k, ts(fp, 128)],
                            rhs=vg[:, k, :],
                            start=(k == 0),
                            stop=(k == 3),
                        )
                    nc.vector.tensor_copy(pt[:, fp, :], ps[:])
                for e_t in range(4):
                    for nh in range(2):
                        ps2 = pAm.tile([128, 512], F32, tag="mp2")
                        for k in range(4):
                            nc.tensor.matmul(
                                ps2[:],
                                lhsT=pt[:, k, ts(e_t, 128)],
                                rhs=ugT[:, k, ts(nh, 512)],
                                start=(k == 0),
                                stop=(k == 3),
                            )
                        nc.vector.tensor_copy(
                            mcat16[:, e_t, g * H + nh * 512 : g * H + (nh + 1) * 512],
                            ps2[:],
                        )

            # interleave the gather+transpose pipeline with the M-build so
            # the PE has matmul work while gathers stream in
            for m in range(MT):
                gather_tile(m)
                if m in (3, 8, 13, 18):
                    mbuild_gate((m - 3) // 5)

            phAt.__exit__(None, None, None)
            phAm.__exit__(None, None, None)
            phAu = tc.tile_pool(name="phAu", bufs=1, space="PSUM")
            pAu = phAu.__enter__()

            # u = embT^T @ M + bias  ->  u_d (bf16)
            for m in range(MT):
                u16 = pA.tile([128, 4 * H], BF16, tag=f"u{m % 2}", name=f"u{m % 2}")
                for n in range(8):
                    pu = pAu.tile([128, 512], F32, tag=f"pu{n % 8}", name=f"pu{n % 8}")
                    for k in range(4):
                        nc.tensor.matmul(
                            pu[:],
                            lhsT=emb16[:, k, ts(m, 128)],
                            rhs=mcat16[:, k, ts(n, 512)],
                            start=(k == 0),
                            stop=(k == 3),
                        )
                    nc.vector.tensor_add(u16[:, ts(n, 512)], pu[:], ub16[:, ts(n, 512)])
                nc.sync.dma_start(u_d[ts(m, 128), :], u16[:])

            phAu.__exit__(None, None, None)
            phA.__exit__(None, None, None)

            # ================= phase B: recurrence + interleaved vocab ======
            with (
                tc.tile_pool(name="phB", bufs=1) as pB,
                tc.tile_pool(name="phBg", bufs=1, space="PSUM") as pBg,
                tc.tile_pool(name="phBv", bufs=1, space="PSUM") as pBv,
                tc.tile_pool(name="phBt", bufs=2, space="PSUM") as pBt,
            ):
                hpair = [
                    pB.tile([128, 8, 128], BF16, tag=f"hp{s}", name=f"hp{s}")
                    for s in range(3)
                ]
                hp8 = [
                    pB.tile([128, 8, 64], FP8, tag=f"h8{s}", name=f"h8{s}")
                    for s in range(2)
                ]
                ut = [
                    pB.tile([64, 4 * H], BF16, tag=f"ut{s}", name=f"ut{s}")
                    for s in range(2)
                ]
                gs = pB.tile([64, 4 * H], F32, tag="gs")
                sig = pB.tile([64, 3 * H], F32, tag="sig")
                th = pB.tile([64, H], BF16, tag="th")
                tmp1 = pB.tile([64, H], F32, tag="tmp1")
                hb16 = pB.tile([64, H], BF16, tag="hb16")
                cst = [
                    pB.tile([64, H], F32, tag=f"cs{s}", name=f"cs{s}") for s in range(2)
                ]

                def load_ut(t):
                    nc.sync.dma_start(ut[t % 2][:], u_d[ts(t, 64), :])

                load_ut(0)
                load_ut(1)

                CT16 = pB.tile([128, 8, VS], BF16, tag="ct16")
                for q in range(4):
                    nc.scalar.dma_start(
                        CT16[:, ts(q, 2), :],
                        CT_d[q * 256 : (q + 1) * 256, :].rearrange(
                            "(ko ki) n -> ki ko n", ki=128
                        ),
                    )
                Cb16 = pB.tile([128, VS], BF16, tag="cb16")
                nc.sync.dma_start(Cb16[:], Cb_d[:])

                def vocab_chunks(m, chunks):
                    hp = hpair[m % 3]
                    for n in chunks:
                        pv = pBv.tile([128, NV], F32, tag=f"v{n % 3}", name=f"v{n % 3}")
                        for k in range(8):
                            nc.tensor.matmul(
                                pv[:],
                                lhsT=hp[:, k, :],
                                rhs=CT16[:, k, ts(n, NV)],
                                start=(k == 0),
                                stop=(k == 7),
                            )
                        pev = pB.tile([128, NV], F32, tag=f"pe{n % 2}", name=f"pe{n % 2}")
                        nc.vector.tensor_add(pev[:], pv[:], Cb16[:, ts(n, NV)])
                        nc.sync.dma_start(out_d[ts(m, 128), ts(n, NV)], pev[:])

                def step(t):
                    parity = t % 2
                    slot = (t // 2) % 3
                    utile = ut[t % 2]
                    # vocab tile for this step (lag-1 pair), split into 2+2
                    # chunk groups inserted to keep the PE busy during the
                    # gate-math chain.
                    vm = (t - 2) // 2 if t >= 2 else None
                    voff = 4 * parity

                    if t == 0:
                        nc.scalar.activation(th[:], utile[:, 3 * H :], TANH)
                        nc.scalar.activation(
                            sig[:, : 2 * H], utile[:, : 2 * H], SIG
                        )
                        nc.scalar.activation(
                            sig[:, 2 * H :], utile[:, 2 * H : 3 * H], SIG
                        )
                    else:
                        h8 = hp8[(t - 1) % 2]
                        for nb in range(8):
                            ga = pBg.tile(
                                [64, 512], F32, tag=f"ga{nb % 2}", name=f"ga{nb % 2}"
                            )
                            for p in range(4):
                                nc.tensor.matmul(
                                    ga[:],
                                    lhsT=h8[:, ts(p, 2), :],
                                    rhs=W8[:, p, :, ts(nb, 512)],
                                    start=(p == 0),
                                    stop=False,
                                    perf_mode=DR,
                                )
                            for p in range(4):
                                nc.tensor.matmul(
                                    ga[:],
                                    lhsT=h8[:, ts(p, 2), :],
                                    rhs=W8lo[:, p, :, ts(nb, 512)],
                                    start=False,
                                    stop=(p == 3),
                                    perf_mode=DR,
                                )
                            # gs = ga/(W_SCALE*H_SCALE) + u
                            nc.vector.scalar_tensor_tensor(
                                gs[:, ts(nb, 512)],
                                ga[:],
                                INV_SCALE,
                                utile[:, ts(nb, 512)],
                                mybir.AluOpType.mult,
                                mybir.AluOpType.add,
                            )
                        nc.scalar.activation(th[:], gs[:, 3 * H :], TANH)
                        nc.scalar.activation(sig[:, : 2 * H], gs[:, : 2 * H], SIG)
                        nc.scalar.activation(
                            sig[:, 2 * H :], gs[:, 2 * H : 3 * H], SIG
                        )

                    # prefetch u for step t+2 — AFTER this step's reads of
                    # ut[t % 2] (same buffer in the 2-ring) are issued
                    if t + 2 < T:
                        load_ut(t + 2)
                    cn = cst[t % 2]
                    cp = cst[1 - t % 2]
                    nc.vector.tensor_mul(tmp1[:], sig[:, :H], th[:])
                    if t == 0:
                        nc.vector.tensor_copy(cn[:], tmp1[:])
                    else:
                        nc.vector.tensor_mul(cn[:], sig[:, H : 2 * H], cp[:])
                        nc.vector.tensor_add(cn[:], cn[:], tmp1[:])
                    nc.vector.tensor_mul(hb16[:], sig[:, 2 * H : 3 * H], cn[:])

                    # first half of the vocab tile work before the h
                    # transposes so the PE has work while sig/c/h resolve
                    if vm is not None:
                        vocab_chunks(vm, range(voff, voff + 2))
                    for k in range(8):
                        tp = pBt.tile([128, 64], BF16, tag="htp")
                        nc.tensor.transpose(tp[:], hb16[:, ts(k, 128)], id64b[:])
                        nc.vector.tensor_copy(hpair[slot][:, k, ts(parity, 64)], tp[:])
                    # fp8 copy of this step's h^T for the next step's DR mms
                    nc.vector.tensor_scalar_mul(
                        hp8[parity][:], hpair[slot][:, :, ts(parity, 64)], H_SCALE
                    )
                    if vm is not None:
                        vocab_chunks(vm, range(voff + 2, voff + 4))

                for t in range(T):
                    step(t)
                vocab_chunks(19, range(8))

    nc.compile()
    return nc


def kernel(**inputs):
    captions = np.asarray(inputs["captions"])
    B_w = np.asarray(inputs["B_w"], dtype=np.float32)
    V_w = np.asarray(inputs["V_w"], dtype=np.float32)
    V_b = np.asarray(inputs["V_b"], dtype=np.float32)
    S_w = np.asarray(inputs["S_w"], dtype=np.float32)
    S_b = np.asarray(inputs["S_b"], dtype=np.float32)
    U_w = np.asarray(inputs["U_w"], dtype=np.float32)
    U_b = np.asarray(inputs["U_b"], dtype=np.float32)
    W_w = np.asarray(inputs["W_w"], dtype=np.float32)
    W_b = np.asarray(inputs["W_b"], dtype=np.float32)
    C_w = np.asarray(inputs["C_w"], dtype=np.float32)
    C_b = np.asarray(inputs["C_b"], dtype=np.float32)

    bf16 = ml_dtypes.bfloat16

    # --- host-side layout prep (weights only) ---
    cap = np.ascontiguousarray(captions.T.reshape(TOK, 1)).astype(np.int32)
    SgT = np.ascontiguousarray(S_w.transpose(0, 2, 1))
    UgT = np.ascontiguousarray(U_w.transpose(0, 2, 1))
    # Wmov: gate-major [K, 4H], scaled, in the fp8 DoubleRow SBUF layout
    # [ki, pair, i, col] where k = 256*pair + 128*i + ki.
    fp8 = ml_dtypes.float8_e4m3

    def dr_layout(a):
        return np.ascontiguousarray(
            a.reshape(4, 2, 128, 4 * H).transpose(2, 0, 1, 3).astype(fp8)
        )

    W_gm = W_w.transpose(2, 0, 1).reshape(H, 4 * H) * W_SCALE
    Wmov = dr_layout(W_gm)
    Wlo = dr_layout(
        W_gm - Wmov.transpose(1, 2, 0, 3).reshape(H, 4 * H).astype(np.float32)
    )
    # gate bias chain, folded: ((V_b @ S^T + S_b) @ U^T + U_b) + W_b
    bs = np.einsum("gf,gof->go", V_b, S_w) + S_b  # [4, F]
    bu = np.einsum("gf,ghf->gh", bs, U_w) + U_b  # [4, H]
    gate_bias = (bu + W_b).reshape(4 * H)  # gate-major [4H]
    ub_rep = np.ascontiguousarray(
        np.broadcast_to(gate_bias, (128, 4 * H)).astype(bf16)
    )
    CT = np.ascontiguousarray(C_w.T)  # [H, V]

    nc = _build()

    in_maps = []
    for c in range(NCORES):
        in_maps.append(
            {
                "cap": cap,
                "Bw": B_w,
                "Vg": V_w,
                "SgT": SgT,
                "UgT": UgT,
                "Wmov": Wmov,
                "Wlo": Wlo,
                "ubias": ub_rep,
                "CT": np.ascontiguousarray(CT[:, c * VS : (c + 1) * VS].astype(bf16)),
                "Cb": np.ascontiguousarray(
                    np.broadcast_to(C_b[c * VS : (c + 1) * VS], (128, VS)).astype(bf16)
                ),
            }
        )

    global _last_in_maps
    _last_in_maps = in_maps

    res = run_bass_kernel_spmd(nc, in_maps, list(range(NCORES)))
    out = np.concatenate([res.results[c]["out"] for c in range(NCORES)], axis=1)
    return out.astype(np.float32)


_last_in_maps = None



# revision 12
# speedup vs baseline: 1.9808x; 1.9808x over previous
"""Trainium2 Bass kernel for nn_DecoderFactoredLSTM (v3).

Factored-LSTM decoder:
  emb = B_w[captions]                       [B,T,E] -> tokens [T*B, E]
  u   = emb @ (V^T S^T U^T) + bias          [T*B, 4H]   (gate pre-activations)
  recurrence over T=40 steps (LSTM, no tanh on c for h)
  out = hiddens @ C_w^T + C_b               [T*B, V]

v3 strategy:
  * Host folds the whole input path into one fused table
        BM = B_w @ (U S V)^T + gate_bias        [V, 4H] bf16
    so the device gathers gate pre-activations u directly per step
    (2 indirect DMAs/step), eliminating phase A matmuls entirely.
  * Recurrence in bf16 with 2-way column-tiled matmuls: each [K=128,
    M=64, N=512] pair runs concurrently on PE column groups 0-1 / 2-3
    (psum partitions 0:64 / 64:128), restoring full-array efficiency
    at batch 64.  Gate/psum layout [128=(half,b), 512] keeps every
    element-wise op and activation full-width.
  * Vocab projection (8-way vocab-sharded, bf16) interleaved into the
    recurrence to fill PE gaps, exactly lag-1-pair like v2.

Column layout: gate-major, split in h-halves:
  psum tile g: partition (half*64 + b), col j  ->  gate g, h = 512*half + j
  u table BMc[v, half, g*512 + j] = BM[v, g*1024 + 512*half + j]
"""

import sys

if "/opt/trn_rl_repo" not in sys.path:
    sys.path.insert(0, "/opt/trn_rl_repo")

import ml_dtypes
import numpy as np

import concourse.bass as bass
import concourse.mybir as mybir
import concourse.tile as tile
from concourse import bacc
from concourse.bass import ts
from concourse.bass_utils import run_bass_kernel_spmd
from concourse.masks import make_identity

B, T, E, H, F, V = 64, 40, 512, 1024, 512, 32000
NCORES = 8
VS = V // NCORES  # vocab slice per core: 4000
TOK = T * B  # 2560 tokens
NV = VS // 8  # 500 vocab cols per chunk
F32 = mybir.dt.float32
BF16 = mybir.dt.bfloat16
I32 = mybir.dt.int32
SIG = mybir.ActivationFunctionType.Sigmoid
TANH = mybir.ActivationFunctionType.Tanh


def _build():
    nc = bacc.Bacc(None, target_bir_lowering=False, debug=False)

    with tile.TileContext(nc) as tc:
        idx_d = nc.declare_dram_parameter("idx", [128, T], I32, isOutput=False)
        # row 2v+half = BM[v, half-block]; idx pre-doubled on host so one
        # 128-partition gather fetches both halves (partitions 64+ get +1)
        BM_d = nc.declare_dram_parameter("BMab", [2 * V, 2048], BF16, isOutput=False)
        W_d = nc.declare_dram_parameter("Wg", [128, 8, 4 * H], BF16, isOutput=False)
        CT_d = nc.declare_dram_parameter("CTr", [128, 8, VS], BF16, isOutput=False)
        Cb_d = nc.declare_dram_parameter("Cb", [128, VS], BF16, isOutput=False)

        out_d = nc.declare_dram_parameter("out", [TOK, VS], F32, isOutput=True)

        with (
            tc.tile_pool(name="const", bufs=1) as const,
            tc.tile_pool(name="pers", bufs=1) as pB,
            tc.tile_pool(name="psg", bufs=1, space="PSUM") as pBg,
            tc.tile_pool(name="psv", bufs=1, space="PSUM") as pBv,
            tc.tile_pool(name="pst", bufs=2, space="PSUM") as pBt,
        ):
            id128b = const.tile([128, 128], BF16, tag="id128b")
            make_identity(nc, id128b)

            idx_s = pB.tile([128, T], I32, tag="idx")
            nc.sync.dma_start(idx_s[:], idx_d[:])

            # u tiles: ring of 4, [128=(half,b), 2048=(g,j)] bf16
            ut = [pB.tile([128, 2048], BF16, tag=f"ut{s}", name=f"ut{s}") for s in range(4)]

            def gather_u(t):
                s = t % 4
                nc.gpsimd.indirect_dma_start(
                    out=ut[s][:],
                    out_offset=None,
                    in_=BM_d[:],
                    in_offset=bass.IndirectOffsetOnAxis(ap=idx_s[:, t : t + 1], axis=0),
                )

            for t in range(3):
                gather_u(t)

            # recurrence weights, k-chunked: [ki, ko, gate-major 4H]
            W_s = pB.tile([128, 8, 4 * H], BF16, tag="wg")
            for k in range(8):
                nc.sync.dma_start(W_s[:, k, :], W_d[:, k, :])

            # vocab weights
            CT16 = pB.tile([128, 8, VS], BF16, tag="ct16")
            for k in range(8):
                nc.scalar.dma_start(CT16[:, k, :], CT_d[:, k, :])
            Cb16 = pB.tile([128, VS], BF16, tag="cb16")
            nc.scalar.dma_start(Cb16[:], Cb_d[:])

            # state
            # hT ring: [128=k-part, 2=half, 4=c, 2=parity, 64=b] bf16
            hTr = [
                pB.tile([128, 2, 4, 2, 64], BF16, tag=f"hT{s}", name=f"hT{s}")
                for s in range(3)
            ]
            gs = pB.tile([128, 2048], BF16, tag="gs")
            sig = pB.tile([128, 1536], BF16, tag="sig")
            th = pB.tile([128, 512], BF16, tag="th")
            tmp1 = pB.tile([128, 512], F32, tag="tmp1")
            h16 = pB.tile([128, 512], BF16, tag="h16")
            cst = [pB.tile([128, 512], F32, tag=f"cs{s}", name=f"cs{s}") for s in range(2)]

            def vocab_chunks(m, chunks):
                hp = hTr[m % 3]
                for n in chunks:
                    pv = pBv.tile([128, NV], F32, tag=f"v{n % 2}", name=f"v{n % 2}")
                    for k in range(8):
                        nc.tensor.matmul(
                            pv[:],
                            lhsT=hp[:, k // 4, k % 4, :, :],
                            rhs=CT16[:, k, ts(n, NV)],
                            start=(k == 0),
                            stop=(k == 7),
                        )
                    pev = pB.tile([128, NV], F32, tag=f"pe{n % 2}", name=f"pe{n % 2}")
                    nc.vector.tensor_add(pev[:], pv[:], Cb16[:, ts(n, NV)])
                    nc.sync.dma_start(out_d[ts(m, 128), ts(n, NV)], pev[:])

            def step(t):
                parity = t % 2
                slot = (t // 2) % 3
                utile = ut[t % 4]
                vm = (t - 2) // 2 if t >= 2 else None
                voff = 4 * parity

                if t == 0:
                    nc.scalar.activation(sig[:, :], utile[:, :1536], SIG)
                    nc.scalar.activation(th[:], utile[:, 1536:], TANH)
                else:
                    hprev = hTr[((t - 1) // 2) % 3]
                    pparity = (t - 1) % 2
                    gp = [
                        pBg.tile([128, 512], F32, tag=f"gp{g}", name=f"gp{g}")
                        for g in range(4)
                    ]
                    for g in range(4):
                        for k in range(8):
                            hk = hprev[:, k // 4, k % 4, pparity, :]
                            # two concurrent col-group MMs (psum partitions
                            # 0:64 / 64:128); the sim's zero-region group
                            # check is partition-base-unaware, skip it
                            nc.tensor.matmul(
                                gp[g][0:64, :],
                                lhsT=hk,
                                rhs=W_s[:, k, g * 1024 : g * 1024 + 512],
                                start=(k == 0),
                                stop=(k == 7),
                                skip_group_check=True,
                            )
                            nc.tensor.matmul(
                                gp[g][64:128, :],
                                lhsT=hk,
                                rhs=W_s[:, k, g * 1024 + 512 : (g + 1) * 1024],
                                start=(k == 0),
                                stop=(k == 7),
                                skip_group_check=True,
                            )
                        # gs = psum + u  (bf16 out)
                        nc.vector.tensor_add(
                            gs[:, ts(g, 512)], gp[g][:], utile[:, ts(g, 512)]
                        )
                        if g < 3:
                            nc.scalar.activation(
                                sig[:, ts(g, 512)], gs[:, ts(g, 512)], SIG
                            )
                        else:
                            nc.scalar.activation(th[:], gs[:, ts(g, 512)], TANH)

                # prefetch u for step t+3 (after this step's reads are issued)
                if t + 3 < T:
                    gather_u(t + 3)

                cn = cst[t % 2]
                cp = cst[1 - t % 2]
                nc.vector.tensor_mul(tmp1[:], sig[:, 0:512], th[:])
                if t == 0:
                    nc.vector.tensor_copy(cn[:], tmp1[:])
                else:
                    nc.vector.tensor_mul(cn[:], sig[:, 512:1024], cp[:])
                    nc.vector.tensor_add(cn[:], cn[:], tmp1[:])
                nc.vector.tensor_mul(h16[:], sig[:, 1024:1536], cn[:])

                # first half of this step's vocab tile while h resolves
                if vm is not None:
                    vocab_chunks(vm, range(voff, voff + 2))
                # transpose h into hT layout: 4x [128,128] PE transposes
                for cc in range(4):
                    tp = pBt.tile([128, 128], BF16, tag="htp")
                    nc.tensor.transpose(tp[:], h16[:, ts(cc, 128)], id128b[:])
                    nc.vector.tensor_copy(hTr[slot][:, :, cc, parity, :], tp[:])
                if vm is not None:
                    vocab_chunks(vm, range(voff + 2, voff + 4))

            for t in range(T):
                step(t)
            vocab_chunks(19, range(8))

    nc.compile()
    return nc


def kernel(**inputs):
    captions = np.asarray(inputs["captions"])
    B_w = np.asarray(inputs["B_w"], dtype=np.float32)
    V_w = np.asarray(inputs["V_w"], dtype=np.float32)
    V_b = np.asarray(inputs["V_b"], dtype=np.float32)
    S_w = np.asarray(inputs["S_w"], dtype=np.float32)
    S_b = np.asarray(inputs["S_b"], dtype=np.float32)
    U_w = np.asarray(inputs["U_w"], dtype=np.float32)
    U_b = np.asarray(inputs["U_b"], dtype=np.float32)
    W_w = np.asarray(inputs["W_w"], dtype=np.float32)
    W_b = np.asarray(inputs["W_b"], dtype=np.float32)
    C_w = np.asarray(inputs["C_w"], dtype=np.float32)
    C_b = np.asarray(inputs["C_b"], dtype=np.float32)

    bf16 = ml_dtypes.bfloat16

    # --- host-side weight prep ---
    # fused input path: BM = B_w @ (U S V)^T + gate_bias   [V, 4H]
    M2 = np.stack([U_w[g] @ S_w[g] @ V_w[g] for g in range(4)])  # [4, H, E]
    bs = np.einsum("gf,gof->go", V_b, S_w) + S_b
    bu = np.einsum("gf,ghf->gh", bs, U_w) + U_b
    gate_bias = bu + W_b  # [4, H]
    BM = B_w @ M2.reshape(4 * H, E).T + gate_bias.reshape(1, 4 * H)  # [V, 4H]
    # [v, half, g*512+j] = BM[v, g*1024+512*half+j]; halves interleaved row-wise
    BMab = np.ascontiguousarray(
        BM.reshape(V, 4, 2, 512).transpose(0, 2, 1, 3).reshape(2 * V, 2048).astype(bf16)
    )

    # W, gate-major cols, k-chunked [ki, ko, 4H]
    Wgm = W_w.transpose(2, 0, 1).reshape(H, 4 * H)
    Wk = np.ascontiguousarray(
        Wgm.reshape(8, 128, 4 * H).transpose(1, 0, 2).astype(bf16)
    )

    # captions doubled: partition p fetches table row 2*cap + (p // 64)
    capBT = captions.astype(np.int32)  # [B, T]
    idx = np.ascontiguousarray(
        np.concatenate([2 * capBT, 2 * capBT + 1], axis=0)
    )  # [128, T]

    CT = C_w.T  # [H, V]

    nc = _build()

    in_maps = []
    for c in range(NCORES):
        CTc = CT[:, c * VS : (c + 1) * VS]
        CTr = np.ascontiguousarray(
            CTc.reshape(8, 128, VS).transpose(1, 0, 2).astype(bf16)
        )
        in_maps.append(
            {
                "idx": idx,
                "BMab": BMab,
                "Wg": Wk,
                "CTr": CTr,
                "Cb": np.ascontiguousarray(
                    np.broadcast_to(C_b[c * VS : (c + 1) * VS], (128, VS)).astype(bf16)
                ),
            }
        )

    global _last_in_maps
    _last_in_maps = in_maps

    res = run_bass_kernel_spmd(nc, in_maps, list(range(NCORES)))
    out = np.concatenate([res.results[c]["out"] for c in range(NCORES)], axis=1)
    return out.astype(np.float32)


_last_in_maps = None
